# revision 46
# baseline (speedup 1.0000x reference)
"""Trainium2 Bass kernel for nn_HarmonicNoiseOscillator.

Math (validated against the CPU reference):

  out = tanh(vm^2 * g(u) + noise * (alpha + beta*vm)),   u = z mod 1
  g(u) = sum_{h=1..8} w_h sin(2*pi*h*u),  alpha = 0.333*S, beta = -0.133*S,
  S = sum(w_h), w = exp(weight)/||exp(weight)||_2;  fs_mask == 1 because
  8 * max(f0) = 3200 < 22050/4.

  - The 256x linear upsample of f0 makes the phase cumsum z decompose into
    per-frame offsets D1 (host, f64) plus a closed-form within-frame prefix
    F*(A,B,C): an fp16 split-product matmul (k=44) reproduces z to ~3e-6.
  - g is evaluated in ONE activation op via a custom piecewise-cubic
    activation table: the `silu` slot of the silu_and_others pwp set is
    rewritten with 256 least-squares cubic buckets of G(x) = g(8(x-1))
    over x in [1,2) -- a single exponent-0 row (higher-exponent rows were
    observed to misbehave on HW). The activation computes
    silu(z*0.125 + 1.0) == g(z mod 1) for z in [0, 5.65) (z < 5.65 holds
    because f0 <= 400 Hz). Table err < 1.5e-3; the table absorbs the
    mod-1 range reduction so no separate frac op is needed.
  - vm^2 and n2 = alpha + beta*vm are BOTH evaluated directly on the
    tensor engine: uv is per-frame so vm = sum U_i c_i is linear in the
    interp weights, hence vm^2 = sum_{ij} U_i U_j c_i c_j is again a
    matmul (k=24) with host-precomputed rhs rows, and n2 is affine (k=13).
    The elementwise tail is then just x2=g*vmsq, q=noise*n2, pre=x2+q,
    tanh -- six engine ops per tile.
  - noise in / out are carried as bf16 (tolerance is 2e-2; bf16 IO costs
    ~1e-3), halving HBM traffic.

Sharding: pure data parallel, 2 of 16 batch rows per core, 8 cores.
"""

import math
import os
import hashlib
import shutil
import struct
import tempfile

import numpy as np

SR = 22050.0
FRAME = 256
NH = 8
N, L = 16, 512
T = L * FRAME  # 131072
NCORES = 8
NPC = N // NCORES  # batch rows per core
P = 128  # SBUF partitions; partition p holds frames 4p..4p+3
FD = 1024  # free dim: 4 frames x 256 samples
SEG = 4  # frames per partition
KZ = 11 * SEG  # z-matmul contraction rows
KQ = 6 * SEG  # vm^2-matmul contraction rows
KN = 3 * SEG + 1  # n2-matmul contraction rows (affine: + alpha row)
KR = 45  # packed operand rows (n2 block lives at partition base 32)

NBKT_LOG2 = 8  # buckets per binade
NBKT = 1 << NBKT_LOG2
SILU_CTL_EXP0 = 21  # pwl_control_base_pos(14) + (0 - exp_offset(-7))

_NC_CACHE = {}
LAST_RESULTS = None  # BassKernelResults of the most recent run (for test.py)


# ----------------------------------------------------------------- host math

def _interp_consts():
    """Input-independent interpolation constants, in f64.

    c1/c2/c3: per-sample blend weights of (prev, cur, next) frame values for
    the 256x linear interpolation; A/B/C: their within-frame prefix sums.
    """
    s = np.arange(FRAME, dtype=np.float64)
    w1 = 0.5 + (s + 0.5) / 256.0
    w2 = (s + 0.5) / 256.0 - 0.5
    c1 = np.where(s < 128, 1.0 - w1, 0.0)
    c2 = np.where(s < 128, w1, 1.0 - w2)
    c3 = np.where(s < 128, 0.0, w2)
    return c1, c2, c3, np.cumsum(c1), np.cumsum(c2), np.cumsum(c3)


def _neighbors(x):
    prev = np.concatenate([x[:, :1], x[:, :-1]], axis=1)
    nxt = np.concatenate([x[:, 1:], x[:, -1:]], axis=1)
    return prev, x, nxt


def _f16_split(v):
    hi = v.astype(np.float16).astype(np.float64)
    lo = (v - hi).astype(np.float16).astype(np.float64)
    return hi, lo


def _windowed_rhs(vecs):
    """[SEG*len(vecs), FD] f64 matrix, vecs[i] repeated in each 256-col
    segment, windowed so row seg*len(vecs)+i is nonzero only in segment."""
    k = len(vecs)
    out = np.zeros((SEG * k, FD), dtype=np.float64)
    for seg in range(SEG):
        for i, v in enumerate(vecs):
            out[seg * k + i, seg * FRAME : (seg + 1) * FRAME] = v
    return out


# -------------------------------------------------------- custom act table

def _g_derivs(w, u, order):
    h = np.arange(1, NH + 1, dtype=np.float64)
    tp = 2.0 * np.pi
    ang = tp * h * np.asarray(u, np.float64)[..., None]
    k = (tp * h) ** order
    b = [np.sin, np.cos, lambda a: -np.sin(a), lambda a: -np.cos(a)][order % 4](ang)
    return (np.asarray(w, np.float64) * k * b).sum(-1)


def _build_bucket_entries(w):
    """[NBKT+1, 8] f32: NBKT least-squares cubic buckets of
    G(x) = g(8*(x-1)) over x in [1,2), plus a constant bucket used for the
    (unreachable) higher exponent rows."""
    width = 1.0 / NBKT
    cheb = np.cos((2 * np.arange(8) + 1) / 16 * np.pi) * (width / 2)
    ent = np.zeros((NBKT + 1, 8), dtype=np.float32)
    for i in range(NBKT):
        x0 = float(np.float32(1.0 + (i + 0.5) * width))
        y = _g_derivs(w, (x0 + cheb - 1.0) * 8.0, 0)
        c = np.polyfit(cheb, y, 3)
        ent[i, 0] = c[3]
        ent[i, 1] = c[2]
        ent[i, 2] = c[1]
        ent[i, 3] = c[0]
        ent[i, 4] = x0
    ent[NBKT, 0] = _g_derivs(w, 0.0, 0)
    ent[NBKT, 4] = 2.0
    return ent


def _patch_pwp_dir(w):
    """Copy the arch pwp dir and rewrite the silu table of silu_and_others
    with 512 cubic buckets of G(x) = g(x-1) on [1,2). Returns (dir, digest)."""
    from neuronxcc.driver.Job import Job
    from neuronxcc.driver.jobs.support.FindActInfo import findActInfoFile

    src = os.path.dirname(findActInfoFile(Job.getPackageDir(), "gen3"))
    ent = _build_bucket_entries(w)
    digest = hashlib.sha256(
        ent.tobytes() + struct.pack("<II", NBKT_LOG2, SILU_CTL_EXP0)
    ).hexdigest()[:12]
    dst = os.path.join(tempfile.gettempdir(), f"pwp_g_{digest}")
    if not os.path.isdir(dst):
        tmp = dst + f".tmp{os.getpid()}"
        if os.path.isdir(tmp):
            shutil.rmtree(tmp)
        shutil.copytree(src, tmp)
        for f in os.listdir(tmp):
            os.chmod(os.path.join(tmp, f), 0o644)
        bkt_path = os.path.join(tmp, "silu_and_others_bkt.bin")
        bkt = bytearray(open(bkt_path, "rb").read())
        bkt[0 : (NBKT + 1) * 32] = ent.tobytes()
        open(bkt_path, "wb").write(bytes(bkt))
        ctl_path = os.path.join(tmp, "silu_and_others_ctrl.bin")
        ctl = bytearray(open(ctl_path, "rb").read())
        w0 = (NBKT_LOG2 << 16) | ((23 - NBKT_LOG2) << 11) | 0
        ctl[SILU_CTL_EXP0 * 32 : SILU_CTL_EXP0 * 32 + 4] = struct.pack("<I", w0)
        wc = (0 << 16) | (23 << 11) | NBKT  # const bucket, exps 1..3
        for e in range(1, 4):
            ctl[(SILU_CTL_EXP0 + e) * 32 : (SILU_CTL_EXP0 + e) * 32 + 4] = (
                struct.pack("<I", wc)
            )
        open(ctl_path, "wb").write(bytes(ctl))
        os.rename(tmp, dst)
    return dst, digest


# --------------------------------------------------------------- bass build

def _build_nc(digest):
    import concourse.bacc as bacc
    import concourse.mybir as mybir
    import concourse.tile as tile
    import concourse.bass as bass

    f32 = mybir.dt.float32
    f16 = mybir.dt.float16
    bf16 = mybir.dt.bfloat16
    Act = mybir.ActivationFunctionType
    Alu = mybir.AluOpType

    nc = bacc.Bacc(
        "TRN2",
        target_bir_lowering=False,
        debug=False,
        num_devices=NCORES,
    )

    # z/q/n2 operand blocks are packed along the free dim so every matmul
    # operand slice starts at a legal SBUF partition base (0/32/64): z at
    # [0:44, block0], q at [0:24, block1], n2 at [32:45, block1].
    noise_d = nc.dram_tensor("noise", [NPC, P, FD], bf16, kind="ExternalInput")
    lhs_d = nc.dram_tensor("lhs", [NPC, KR, 2 * P], f16, kind="ExternalInput")
    rhs_d = nc.dram_tensor("rhs", [KR, 2 * FD], f16, kind="ExternalInput")
    # table-digest marker: forces a distinct BIR (and thus NEFF cache key)
    # per activation-table contents.
    dig_d = nc.dram_tensor(f"tdig_{digest}", [1, 8], f32, kind="ExternalInput")
    out_d = nc.dram_tensor("out", [NPC, P, FD], bf16, kind="ExternalOutput")

    with tile.TileContext(nc, pool_alloc_mode="queue") as tc:
        with (
            tc.tile_pool(name="const", bufs=1) as cpool,
            tc.tile_pool(name="psum", bufs=2, space=bass.MemorySpace.PSUM) as psum,
        ):
            # compute-gating operands (rhs, lhs) go on the gpsimd DMA queue,
            # bulk noise on the Sync queue; the Scalar queue stays DMA-free
            # so the act-table load is emitted exactly once, early.
            rhs_t = cpool.tile([KR, 2 * FD], f16)
            nc.gpsimd.dma_start(rhs_t[:, 0:FD], rhs_d[:, 0:FD])
            nc.sync.dma_start(rhs_t[:, FD : 2 * FD], rhs_d[:, FD : 2 * FD])
            dig_t = cpool.tile([1, 8], f32)
            lhs_ts = []
            noise_ts = []
            for n in range(NPC):
                lhs_ts.append(cpool.tile([KR, 2 * P], f16, name=f"lhs{n}"))
                nc.gpsimd.dma_start(lhs_ts[n][:], lhs_d[n])
                noise_ts.append(cpool.tile([P, FD], bf16, name=f"noise{n}"))
                nc.sync.dma_start(noise_ts[n][:], noise_d[n])
            nc.sync.dma_start(dig_t[:], dig_d[:])

            HF = 512  # pipeline-unit free dim (one PSUM bank)
            for n in range(NPC):
                pool_cm = tc.tile_pool(name=f"work{n}", bufs=2)
                pool = pool_cm.__enter__()
                lhs_t = lhs_ts[n]
                noise_t = noise_ts[n]
                o_t = pool.tile([P, FD], bf16, tag="o")

                for half in range(2):
                    cols = bass.ts(half, HF)

                    # z = phase cycles; vmsq = vm^2; n2 = alpha + beta*vm
                    z_p = psum.tile([P, HF], f32, tag="z")
                    nc.tensor.matmul(
                        z_p[:], lhs_t[0:KZ, 0:P], rhs_t[0:KZ, cols]
                    )
                    q_p = psum.tile([P, HF], f32, tag="q")
                    nc.tensor.matmul(
                        q_p[:],
                        lhs_t[0:KQ, P : 2 * P],
                        rhs_t[0:KQ, bass.ts(2 + half, HF)],
                    )
                    n2_p = psum.tile([P, HF], f32, tag="n2")
                    nc.tensor.matmul(
                        n2_p[:],
                        lhs_t[32 : 32 + KN, P : 2 * P],
                        rhs_t[32 : 32 + KN, bass.ts(2 + half, HF)],
                    )

                    # g(z mod 1) via the patched silu table: silu(z/8 + 1)
                    g_t = pool.tile([P, HF], bf16, tag="g")
                    nc.scalar.activation(
                        g_t[:], z_p[:], Act.Silu, bias=1.0, scale=0.125
                    )

                    # pre = g*vmsq + noise*n2 (in place: g <- g*q;
                    # noise <- noise*n2; g <- g + noise)
                    nc.vector.tensor_mul(g_t[:], g_t[:], q_p[:])
                    nc.vector.tensor_mul(
                        noise_t[:, cols], noise_t[:, cols], n2_p[:]
                    )
                    nc.gpsimd.tensor_add(g_t[:], g_t[:], noise_t[:, cols])
                    nc.scalar.activation(o_t[:, cols], g_t[:], Act.Tanh)
                nc.sync.dma_start(out_d[n], o_t[:])
                pool_cm.__exit__(None, None, None)

    nc.compile()
    return nc


# ------------------------------------------------------------------- driver

def _host_inputs(f0, uv, weight, noise, alpha, beta):
    """Build the per-core input maps (all host math in f64)."""
    import ml_dtypes

    f0 = np.asarray(f0, np.float64).reshape(N, L)
    uv = np.asarray(uv, np.float64).reshape(N, L)
    noise_bf = np.ascontiguousarray(
        np.asarray(noise, np.float32).reshape(N, T)
    ).astype(ml_dtypes.bfloat16)

    c1, c2, c3, A, B, C = _interp_consts()
    Fp, Fc, Fn = _neighbors(f0 / SR)
    Up, Uc, Un = _neighbors(uv)

    # per-frame phase offsets (cycles), f64 exact then frac
    FS = Fp * A[-1] + Fc * B[-1] + Fn * C[-1]  # frame sums of f0_up/SR
    C0 = np.concatenate([np.zeros((N, 1)), np.cumsum(FS, axis=1)[:, :-1]], axis=1)
    D1 = np.mod(C0, 1.0)

    A1, A2 = _f16_split(A)
    B1, B2 = _f16_split(B)
    C1v, C2v = _f16_split(C)
    ones = np.ones(FRAME)

    # rhs rows per seg: [A1,B1,C1, A2,B2,C2, A1,B1,C1, 1, 1] pairing with
    # lhs  rows        [F1p,F1c,F1n, F1p,F1c,F1n, F2p,F2c,F2n, D11, D12]
    rhs_z = _windowed_rhs(
        [A1, B1, C1v, A2, B2, C2v, A1, B1, C1v, ones, ones]
    ).astype(np.float16)
    # vm^2 quadratic form: rhs [c1^2,c2^2,c3^2, c1c2,c1c3,c2c3] against
    # lhs [Up^2,Uc^2,Un^2, 2UpUc,2UpUn,2UcUn]
    rhs_q = _windowed_rhs(
        [c1 * c1, c2 * c2, c3 * c3, c1 * c2, c1 * c3, c2 * c3]
    ).astype(np.float16)
    # n2 = alpha + beta*vm: windowed beta*c rows + one global alpha row
    rhs_n2 = np.concatenate(
        [
            _windowed_rhs([beta * c1, beta * c2, beta * c3]),
            np.full((1, FD), alpha),
        ]
    ).astype(np.float16)

    F1p, F2p = _f16_split(Fp)
    F1c, F2c = _f16_split(Fc)
    F1n, F2n = _f16_split(Fn)
    D11, D12 = _f16_split(D1)

    # pack z/q/n2 blocks along the free dim (n2 at partition base 32)
    rhs = np.zeros((KR, 2 * FD), dtype=np.float16)
    rhs[:KZ, 0:FD] = rhs_z
    rhs[:KQ, FD : 2 * FD] = rhs_q
    rhs[32 : 32 + KN, FD : 2 * FD] = rhs_n2

    jidx = 4 * np.arange(P)[None, :] + np.arange(SEG)[:, None]  # [SEG, P]
    zrows = [F1p, F1c, F1n, F1p, F1c, F1n, F2p, F2c, F2n, D11, D12]
    qrows = [Up * Up, Uc * Uc, Un * Un, 2 * Up * Uc, 2 * Up * Un, 2 * Uc * Un]
    nrows = [Up, Uc, Un]
    in_maps = []
    for core in range(NCORES):
        rows = range(core * NPC, (core + 1) * NPC)
        lhs = np.zeros((NPC, KR, 2 * P), dtype=np.float16)
        lhs[:, 32 + KN - 1, P : 2 * P] = 1.0  # alpha row
        for ln, nr in enumerate(rows):
            for seg in range(SEG):
                j = jidx[seg]
                for i, r in enumerate(zrows):
                    lhs[ln, seg * 11 + i, 0:P] = r[nr, j].astype(np.float16)
                for i, r in enumerate(qrows):
                    lhs[ln, seg * 6 + i, P : 2 * P] = r[nr, j].astype(np.float16)
                for i, r in enumerate(nrows):
                    lhs[ln, 32 + seg * 3 + i, P : 2 * P] = r[nr, j].astype(
                        np.float16
                    )
        in_maps.append(
            {
                "noise": noise_bf[core * NPC : (core + 1) * NPC].reshape(NPC, P, FD),
                "lhs": lhs,
                "rhs": rhs,
            }
        )
    return in_maps


def kernel(f0, uv, weight, noise):
    global LAST_RESULTS
    from concourse.bass_utils import run_bass_kernel_spmd

    weight = np.asarray(weight, np.float64).reshape(NH)
    w = np.exp(weight)
    w = w / max(np.sqrt((w * w).sum()), 1e-12)
    S = float(w.sum())
    alpha = float(np.float32(0.333 * S))
    beta = float(np.float32((0.2 - 0.333) * S))

    pwp_dir, digest = _patch_pwp_dir(w)
    os.environ["BASS_ACT_ROOT_JSON_PATH"] = os.path.join(pwp_dir, "act_info.json")

    key = digest
    if key not in _NC_CACHE:
        _NC_CACHE[key] = _build_nc(digest)
    nc = _NC_CACHE[key]

    in_maps = _host_inputs(f0, uv, weight, noise, alpha, beta)
    dig = np.zeros((1, 8), dtype=np.float32)
    dig[0, :] = np.frombuffer(
        hashlib.sha256(digest.encode()).digest()[:32], dtype=np.float32
    )[:8]
    for m in in_maps:
        m[f"tdig_{digest}"] = dig

    res = run_bass_kernel_spmd(nc, in_maps, list(range(NCORES)))
    LAST_RESULTS = res
    out = np.empty((N, 1, T), dtype=np.float32)
    for core in range(NCORES):
        out[core * NPC : (core + 1) * NPC, 0, :] = (
            res.results[core]["out"].astype(np.float32).reshape(NPC, T)
        )
    return out


# revision 49
# speedup vs baseline: 1.0921x; 1.0921x over previous
"""Trainium2 Bass kernel for nn_HarmonicNoiseOscillator.

Math (validated against the CPU reference):

  out = tanh(vm^2 * g(u) + noise * (alpha + beta*vm)),   u = z mod 1
  g(u) = sum_{h=1..8} w_h sin(2*pi*h*u),  alpha = 0.333*S, beta = -0.133*S,
  S = sum(w_h), w = exp(weight)/||exp(weight)||_2;  fs_mask == 1 because
  8 * max(f0) = 3200 < 22050/4.

  - The 256x linear upsample of f0 makes the phase cumsum z decompose into
    per-frame offsets D1 (host, f64) plus a closed-form within-frame prefix
    F*(A,B,C): an fp16 split-product matmul (k=44) reproduces z to ~3e-6.
  - g is evaluated in ONE activation op via a custom piecewise-cubic
    activation table: the `silu` slot of the silu_and_others pwp set is
    rewritten with 256 least-squares cubic buckets of G(x) = g(8(x-1))
    over x in [1,2) -- a single exponent-0 row (higher-exponent rows were
    observed to misbehave on HW). The activation computes
    silu(z*0.125 + 1.0) == g(z mod 1) for z in [0, 5.65) (z < 5.65 holds
    because f0 <= 400 Hz). Table err < 1.5e-3; the table absorbs the
    mod-1 range reduction so no separate frac op is needed.
  - vm^2 and n2 = alpha + beta*vm are BOTH evaluated directly on the
    tensor engine: uv is per-frame so vm = sum U_i c_i is linear in the
    interp weights, hence vm^2 = sum_{ij} U_i U_j c_i c_j is again a
    matmul (k=24) with host-precomputed rhs rows, and n2 is affine (k=13).
    The elementwise tail is then just x2=g*vmsq, q=noise*n2, pre=x2+q,
    tanh -- six engine ops per tile.
  - noise in / out are carried as bf16 (tolerance is 2e-2; bf16 IO costs
    ~1e-3), halving HBM traffic.

Sharding: pure data parallel, 2 of 16 batch rows per core, 8 cores.
"""

import math
import os
import hashlib
import shutil
import struct
import tempfile

import numpy as np

SR = 22050.0
FRAME = 256
NH = 8
N, L = 16, 512
T = L * FRAME  # 131072
NCORES = 8
NPC = N // NCORES  # batch rows per core
P = 128  # SBUF partitions; partition p holds frames 4p..4p+3
FD = 1024  # free dim: 4 frames x 256 samples
SEG = 4  # frames per partition
KZ = 11 * SEG  # z-matmul contraction rows
KQ = 6 * SEG  # vm^2-matmul contraction rows
KN = 3 * SEG + 1  # n2-matmul contraction rows (affine: + alpha row)
KR = 45  # packed operand rows (n2 block lives at partition base 32)

NBKT_LOG2 = 8  # buckets per binade
NBKT = 1 << NBKT_LOG2
SILU_CTL_EXP0 = 21  # pwl_control_base_pos(14) + (0 - exp_offset(-7))

_NC_CACHE = {}
LAST_RESULTS = None  # BassKernelResults of the most recent run (for test.py)


# ----------------------------------------------------------------- host math

def _interp_consts():
    """Input-independent interpolation constants, in f64.

    c1/c2/c3: per-sample blend weights of (prev, cur, next) frame values for
    the 256x linear interpolation; A/B/C: their within-frame prefix sums.
    """
    s = np.arange(FRAME, dtype=np.float64)
    w1 = 0.5 + (s + 0.5) / 256.0
    w2 = (s + 0.5) / 256.0 - 0.5
    c1 = np.where(s < 128, 1.0 - w1, 0.0)
    c2 = np.where(s < 128, w1, 1.0 - w2)
    c3 = np.where(s < 128, 0.0, w2)
    return c1, c2, c3, np.cumsum(c1), np.cumsum(c2), np.cumsum(c3)


def _neighbors(x):
    prev = np.concatenate([x[:, :1], x[:, :-1]], axis=1)
    nxt = np.concatenate([x[:, 1:], x[:, -1:]], axis=1)
    return prev, x, nxt


def _f16_split(v):
    hi = v.astype(np.float16).astype(np.float64)
    lo = (v - hi).astype(np.float16).astype(np.float64)
    return hi, lo


def _windowed_rhs(vecs):
    """[SEG*len(vecs), FD] f64 matrix, vecs[i] repeated in each 256-col
    segment, windowed so row seg*len(vecs)+i is nonzero only in segment."""
    k = len(vecs)
    out = np.zeros((SEG * k, FD), dtype=np.float64)
    for seg in range(SEG):
        for i, v in enumerate(vecs):
            out[seg * k + i, seg * FRAME : (seg + 1) * FRAME] = v
    return out


# -------------------------------------------------------- custom act table

def _g_derivs(w, u, order):
    h = np.arange(1, NH + 1, dtype=np.float64)
    tp = 2.0 * np.pi
    ang = tp * h * np.asarray(u, np.float64)[..., None]
    k = (tp * h) ** order
    b = [np.sin, np.cos, lambda a: -np.sin(a), lambda a: -np.cos(a)][order % 4](ang)
    return (np.asarray(w, np.float64) * k * b).sum(-1)


def _build_bucket_entries(w):
    """[NBKT+1, 8] f32: NBKT least-squares cubic buckets of
    G(x) = g(8*(x-1)) over x in [1,2), plus a constant bucket used for the
    (unreachable) higher exponent rows."""
    width = 1.0 / NBKT
    cheb = np.cos((2 * np.arange(8) + 1) / 16 * np.pi) * (width / 2)
    ent = np.zeros((NBKT + 1, 8), dtype=np.float32)
    for i in range(NBKT):
        x0 = float(np.float32(1.0 + (i + 0.5) * width))
        y = _g_derivs(w, (x0 + cheb - 1.0) * 8.0, 0)
        c = np.polyfit(cheb, y, 3)
        ent[i, 0] = c[3]
        ent[i, 1] = c[2]
        ent[i, 2] = c[1]
        ent[i, 3] = c[0]
        ent[i, 4] = x0
    ent[NBKT, 0] = _g_derivs(w, 0.0, 0)
    ent[NBKT, 4] = 2.0
    return ent


def _patch_pwp_dir(w):
    """Copy the arch pwp dir and rewrite the silu table of silu_and_others
    with 512 cubic buckets of G(x) = g(x-1) on [1,2). Returns (dir, digest)."""
    from neuronxcc.driver.Job import Job
    from neuronxcc.driver.jobs.support.FindActInfo import findActInfoFile

    src = os.path.dirname(findActInfoFile(Job.getPackageDir(), "gen3"))
    ent = _build_bucket_entries(w)
    digest = hashlib.sha256(
        ent.tobytes() + struct.pack("<II", NBKT_LOG2, SILU_CTL_EXP0)
    ).hexdigest()[:12]
    dst = os.path.join(tempfile.gettempdir(), f"pwp_g_{digest}")
    if not os.path.isdir(dst):
        tmp = dst + f".tmp{os.getpid()}"
        if os.path.isdir(tmp):
            shutil.rmtree(tmp)
        shutil.copytree(src, tmp)
        for f in os.listdir(tmp):
            os.chmod(os.path.join(tmp, f), 0o644)
        bkt_path = os.path.join(tmp, "silu_and_others_bkt.bin")
        bkt = bytearray(open(bkt_path, "rb").read())
        bkt[0 : (NBKT + 1) * 32] = ent.tobytes()
        open(bkt_path, "wb").write(bytes(bkt))
        ctl_path = os.path.join(tmp, "silu_and_others_ctrl.bin")
        ctl = bytearray(open(ctl_path, "rb").read())
        w0 = (NBKT_LOG2 << 16) | ((23 - NBKT_LOG2) << 11) | 0
        ctl[SILU_CTL_EXP0 * 32 : SILU_CTL_EXP0 * 32 + 4] = struct.pack("<I", w0)
        wc = (0 << 16) | (23 << 11) | NBKT  # const bucket, exps 1..3
        for e in range(1, 4):
            ctl[(SILU_CTL_EXP0 + e) * 32 : (SILU_CTL_EXP0 + e) * 32 + 4] = (
                struct.pack("<I", wc)
            )
        open(ctl_path, "wb").write(bytes(ctl))
        os.rename(tmp, dst)
    return dst, digest


# --------------------------------------------------------------- bass build

def _build_nc(digest):
    import concourse.bacc as bacc
    import concourse.mybir as mybir
    import concourse.tile as tile
    import concourse.bass as bass

    f32 = mybir.dt.float32
    f16 = mybir.dt.float16
    bf16 = mybir.dt.bfloat16
    Act = mybir.ActivationFunctionType
    Alu = mybir.AluOpType

    nc = bacc.Bacc(
        "TRN2",
        target_bir_lowering=False,
        debug=False,
        num_devices=NCORES,
    )

    # z/q/n2 operand blocks are packed along the free dim so every matmul
    # operand slice starts at a legal SBUF partition base (0/32/64): z at
    # [0:44, block0], q at [0:24, block1], n2 at [32:45, block1].
    noise_d = nc.dram_tensor("noise", [NPC, P, FD], bf16, kind="ExternalInput")
    lhs_d = nc.dram_tensor("lhs", [NPC, KR, 2 * P], f16, kind="ExternalInput")
    rhs_d = nc.dram_tensor("rhs", [KR, 2 * FD], f16, kind="ExternalInput")
    # table-digest marker: forces a distinct BIR (and thus NEFF cache key)
    # per activation-table contents.
    dig_d = nc.dram_tensor(f"tdig_{digest}", [1, 8], f32, kind="ExternalInput")
    out_d = nc.dram_tensor("out", [NPC, P, FD], bf16, kind="ExternalOutput")

    with tile.TileContext(nc) as tc:
        with (
            tc.tile_pool(name="const", bufs=1) as cpool,
            tc.tile_pool(name="work", bufs=3) as pool,
            tc.tile_pool(name="psum", bufs=2, space=bass.MemorySpace.PSUM) as psum,
        ):
            # compute-gating operands (rhs, lhs) go on the gpsimd DMA queue,
            # bulk noise on the Sync queue; the Scalar queue stays DMA-free
            # so the act-table load is emitted exactly once, early.
            rhs_t = cpool.tile([KR, 2 * FD], f16)
            nc.gpsimd.dma_start(rhs_t[:, 0:FD], rhs_d[:, 0:FD])
            nc.sync.dma_start(rhs_t[:, FD : 2 * FD], rhs_d[:, FD : 2 * FD])
            dig_t = cpool.tile([1, 8], f32)
            lhs_ts = []
            noise_ts = []
            for n in range(NPC):
                lhs_ts.append(cpool.tile([KR, 2 * P], f16, name=f"lhs{n}"))
                nc.gpsimd.dma_start(lhs_ts[n][:], lhs_d[n])
                noise_ts.append(cpool.tile([P, FD], bf16, name=f"noise{n}"))
                nc.sync.dma_start(noise_ts[n][:], noise_d[n])
            nc.sync.dma_start(dig_t[:], dig_d[:])

            HF = 512  # pipeline-unit free dim (one PSUM bank)
            for n in range(NPC):
                lhs_t = lhs_ts[n]
                noise_t = noise_ts[n]
                o_t = pool.tile([P, FD], bf16, tag="o")

                for half in range(2):
                    cols = bass.ts(half, HF)

                    # z = phase cycles; vmsq = vm^2; n2 = alpha + beta*vm
                    z_p = psum.tile([P, HF], f32, tag="z")
                    nc.tensor.matmul(
                        z_p[:], lhs_t[0:KZ, 0:P], rhs_t[0:KZ, cols]
                    )
                    q_p = psum.tile([P, HF], f32, tag="q")
                    nc.tensor.matmul(
                        q_p[:],
                        lhs_t[0:KQ, P : 2 * P],
                        rhs_t[0:KQ, bass.ts(2 + half, HF)],
                    )
                    n2_p = psum.tile([P, HF], f32, tag="n2")
                    nc.tensor.matmul(
                        n2_p[:],
                        lhs_t[32 : 32 + KN, P : 2 * P],
                        rhs_t[32 : 32 + KN, bass.ts(2 + half, HF)],
                    )

                    # g(z mod 1) via the patched silu table: silu(z/8 + 1)
                    g_t = pool.tile([P, HF], bf16, tag="g")
                    nc.scalar.activation(
                        g_t[:], z_p[:], Act.Silu, bias=1.0, scale=0.125
                    )

                    # pre = g*vmsq + noise*n2 (in place: g <- g*q;
                    # noise <- noise*n2; g <- g + noise)
                    nc.vector.tensor_mul(g_t[:], g_t[:], q_p[:])
                    nc.vector.tensor_mul(
                        noise_t[:, cols], noise_t[:, cols], n2_p[:]
                    )
                    nc.gpsimd.tensor_add(g_t[:], g_t[:], noise_t[:, cols])
                    nc.scalar.activation(o_t[:, cols], g_t[:], Act.Tanh)
                nc.sync.dma_start(out_d[n], o_t[:])

    nc.compile()
    return nc


# ------------------------------------------------------------------- driver

def _host_inputs(f0, uv, weight, noise, alpha, beta):
    """Build the per-core input maps (all host math in f64)."""
    import ml_dtypes

    f0 = np.asarray(f0, np.float64).reshape(N, L)
    uv = np.asarray(uv, np.float64).reshape(N, L)
    noise_bf = np.ascontiguousarray(
        np.asarray(noise, np.float32).reshape(N, T)
    ).astype(ml_dtypes.bfloat16)

    c1, c2, c3, A, B, C = _interp_consts()
    Fp, Fc, Fn = _neighbors(f0 / SR)
    Up, Uc, Un = _neighbors(uv)

    # per-frame phase offsets (cycles), f64 exact then frac
    FS = Fp * A[-1] + Fc * B[-1] + Fn * C[-1]  # frame sums of f0_up/SR
    C0 = np.concatenate([np.zeros((N, 1)), np.cumsum(FS, axis=1)[:, :-1]], axis=1)
    D1 = np.mod(C0, 1.0)

    A1, A2 = _f16_split(A)
    B1, B2 = _f16_split(B)
    C1v, C2v = _f16_split(C)
    ones = np.ones(FRAME)

    # rhs rows per seg: [A1,B1,C1, A2,B2,C2, A1,B1,C1, 1, 1] pairing with
    # lhs  rows        [F1p,F1c,F1n, F1p,F1c,F1n, F2p,F2c,F2n, D11, D12]
    rhs_z = _windowed_rhs(
        [A1, B1, C1v, A2, B2, C2v, A1, B1, C1v, ones, ones]
    ).astype(np.float16)
    # vm^2 quadratic form: rhs [c1^2,c2^2,c3^2, c1c2,c1c3,c2c3] against
    # lhs [Up^2,Uc^2,Un^2, 2UpUc,2UpUn,2UcUn]
    rhs_q = _windowed_rhs(
        [c1 * c1, c2 * c2, c3 * c3, c1 * c2, c1 * c3, c2 * c3]
    ).astype(np.float16)
    # n2 = alpha + beta*vm: windowed beta*c rows + one global alpha row
    rhs_n2 = np.concatenate(
        [
            _windowed_rhs([beta * c1, beta * c2, beta * c3]),
            np.full((1, FD), alpha),
        ]
    ).astype(np.float16)

    F1p, F2p = _f16_split(Fp)
    F1c, F2c = _f16_split(Fc)
    F1n, F2n = _f16_split(Fn)
    D11, D12 = _f16_split(D1)

    # pack z/q/n2 blocks along the free dim (n2 at partition base 32)
    rhs = np.zeros((KR, 2 * FD), dtype=np.float16)
    rhs[:KZ, 0:FD] = rhs_z
    rhs[:KQ, FD : 2 * FD] = rhs_q
    rhs[32 : 32 + KN, FD : 2 * FD] = rhs_n2

    jidx = 4 * np.arange(P)[None, :] + np.arange(SEG)[:, None]  # [SEG, P]
    zrows = [F1p, F1c, F1n, F1p, F1c, F1n, F2p, F2c, F2n, D11, D12]
    qrows = [Up * Up, Uc * Uc, Un * Un, 2 * Up * Uc, 2 * Up * Un, 2 * Uc * Un]
    nrows = [Up, Uc, Un]
    in_maps = []
    for core in range(NCORES):
        rows = range(core * NPC, (core + 1) * NPC)
        lhs = np.zeros((NPC, KR, 2 * P), dtype=np.float16)
        lhs[:, 32 + KN - 1, P : 2 * P] = 1.0  # alpha row
        for ln, nr in enumerate(rows):
            for seg in range(SEG):
                j = jidx[seg]
                for i, r in enumerate(zrows):
                    lhs[ln, seg * 11 + i, 0:P] = r[nr, j].astype(np.float16)
                for i, r in enumerate(qrows):
                    lhs[ln, seg * 6 + i, P : 2 * P] = r[nr, j].astype(np.float16)
                for i, r in enumerate(nrows):
                    lhs[ln, 32 + seg * 3 + i, P : 2 * P] = r[nr, j].astype(
                        np.float16
                    )
        in_maps.append(
            {
                "noise": noise_bf[core * NPC : (core + 1) * NPC].reshape(NPC, P, FD),
                "lhs": lhs,
                "rhs": rhs,
            }
        )
    return in_maps


def kernel(f0, uv, weight, noise):
    global LAST_RESULTS
    from concourse.bass_utils import run_bass_kernel_spmd

    weight = np.asarray(weight, np.float64).reshape(NH)
    w = np.exp(weight)
    w = w / max(np.sqrt((w * w).sum()), 1e-12)
    S = float(w.sum())
    alpha = float(np.float32(0.333 * S))
    beta = float(np.float32((0.2 - 0.333) * S))

    pwp_dir, digest = _patch_pwp_dir(w)
    os.environ["BASS_ACT_ROOT_JSON_PATH"] = os.path.join(pwp_dir, "act_info.json")

    key = digest
    if key not in _NC_CACHE:
        _NC_CACHE[key] = _build_nc(digest)
    nc = _NC_CACHE[key]

    in_maps = _host_inputs(f0, uv, weight, noise, alpha, beta)
    dig = np.zeros((1, 8), dtype=np.float32)
    dig[0, :] = np.frombuffer(
        hashlib.sha256(digest.encode()).digest()[:32], dtype=np.float32
    )[:8]
    for m in in_maps:
        m[f"tdig_{digest}"] = dig

    res = run_bass_kernel_spmd(nc, in_maps, list(range(NCORES)))
    LAST_RESULTS = res
    out = np.empty((N, 1, T), dtype=np.float32)
    for core in range(NCORES):
        out[core * NPC : (core + 1) * NPC, 0, :] = (
            res.results[core]["out"].astype(np.float32).reshape(NPC, T)
        )
    return out


# revision 50
# speedup vs baseline: 1.1460x; 1.0493x over previous
"""Trainium2 Bass kernel for nn_HarmonicNoiseOscillator.

Math (validated against the CPU reference):

  out = tanh(vm^2 * g(u) + noise * (alpha + beta*vm)),   u = z mod 1
  g(u) = sum_{h=1..8} w_h sin(2*pi*h*u),  alpha = 0.333*S, beta = -0.133*S,
  S = sum(w_h), w = exp(weight)/||exp(weight)||_2;  fs_mask == 1 because
  8 * max(f0) = 3200 < 22050/4.

  - The 256x linear upsample of f0 makes the phase cumsum z decompose into
    per-frame offsets D1 (host, f64) plus a closed-form within-frame prefix
    F*(A,B,C): an fp16 split-product matmul (k=44) reproduces z to ~3e-6.
  - g is evaluated in ONE activation op via a custom piecewise-cubic
    activation table: the `silu` slot of the silu_and_others pwp set is
    rewritten with 256 least-squares cubic buckets of G(x) = g(8(x-1))
    over x in [1,2) -- a single exponent-0 row (higher-exponent rows were
    observed to misbehave on HW). The activation computes
    silu(z*0.125 + 1.0) == g(z mod 1) for z in [0, 5.65) (z < 5.65 holds
    because f0 <= 400 Hz). Table err < 1.5e-3; the table absorbs the
    mod-1 range reduction so no separate frac op is needed.
  - vm^2 and n2 = alpha + beta*vm are BOTH evaluated directly on the
    tensor engine: uv is per-frame so vm = sum U_i c_i is linear in the
    interp weights, hence vm^2 = sum_{ij} U_i U_j c_i c_j is again a
    matmul (k=24) with host-precomputed rhs rows, and n2 is affine (k=13).
    The elementwise tail is then just x2=g*vmsq, q=noise*n2, pre=x2+q,
    tanh -- six engine ops per tile.
  - noise in / out are carried as bf16 (tolerance is 2e-2; bf16 IO costs
    ~1e-3), halving HBM traffic.

Sharding: pure data parallel, 2 of 16 batch rows per core, 8 cores.
"""

import math
import os
import hashlib
import shutil
import struct
import tempfile

import numpy as np

SR = 22050.0
FRAME = 256
NH = 8
N, L = 16, 512
T = L * FRAME  # 131072
NCORES = 8
NPC = N // NCORES  # batch rows per core
P = 128  # SBUF partitions; partition p holds frames 4p..4p+3
FD = 1024  # free dim: 4 frames x 256 samples
SEG = 4  # frames per partition
KZ = 11 * SEG  # z-matmul contraction rows
KQ = 6 * SEG  # vm^2-matmul contraction rows
KN = 3 * SEG + 1  # n2-matmul contraction rows (affine: + alpha row)
KR = 45  # packed operand rows (n2 block lives at partition base 32)

NBKT_LOG2 = 8  # buckets per binade
NBKT = 1 << NBKT_LOG2
SILU_CTL_EXP0 = 21  # pwl_control_base_pos(14) + (0 - exp_offset(-7))

_NC_CACHE = {}
LAST_RESULTS = None  # BassKernelResults of the most recent run (for test.py)


# ----------------------------------------------------------------- host math

def _interp_consts():
    """Input-independent interpolation constants, in f64.

    c1/c2/c3: per-sample blend weights of (prev, cur, next) frame values for
    the 256x linear interpolation; A/B/C: their within-frame prefix sums.
    """
    s = np.arange(FRAME, dtype=np.float64)
    w1 = 0.5 + (s + 0.5) / 256.0
    w2 = (s + 0.5) / 256.0 - 0.5
    c1 = np.where(s < 128, 1.0 - w1, 0.0)
    c2 = np.where(s < 128, w1, 1.0 - w2)
    c3 = np.where(s < 128, 0.0, w2)
    return c1, c2, c3, np.cumsum(c1), np.cumsum(c2), np.cumsum(c3)


def _neighbors(x):
    prev = np.concatenate([x[:, :1], x[:, :-1]], axis=1)
    nxt = np.concatenate([x[:, 1:], x[:, -1:]], axis=1)
    return prev, x, nxt


def _f16_split(v):
    hi = v.astype(np.float16).astype(np.float64)
    lo = (v - hi).astype(np.float16).astype(np.float64)
    return hi, lo


def _windowed_rhs(vecs):
    """[SEG*len(vecs), FD] f64 matrix, vecs[i] repeated in each 256-col
    segment, windowed so row seg*len(vecs)+i is nonzero only in segment."""
    k = len(vecs)
    out = np.zeros((SEG * k, FD), dtype=np.float64)
    for seg in range(SEG):
        for i, v in enumerate(vecs):
            out[seg * k + i, seg * FRAME : (seg + 1) * FRAME] = v
    return out


# -------------------------------------------------------- custom act table

def _g_derivs(w, u, order):
    h = np.arange(1, NH + 1, dtype=np.float64)
    tp = 2.0 * np.pi
    ang = tp * h * np.asarray(u, np.float64)[..., None]
    k = (tp * h) ** order
    b = [np.sin, np.cos, lambda a: -np.sin(a), lambda a: -np.cos(a)][order % 4](ang)
    return (np.asarray(w, np.float64) * k * b).sum(-1)


def _build_bucket_entries(w):
    """[NBKT+1, 8] f32: NBKT least-squares cubic buckets of
    G(x) = g(8*(x-1)) over x in [1,2), plus a constant bucket used for the
    (unreachable) higher exponent rows."""
    width = 1.0 / NBKT
    cheb = np.cos((2 * np.arange(8) + 1) / 16 * np.pi) * (width / 2)
    ent = np.zeros((NBKT + 1, 8), dtype=np.float32)
    for i in range(NBKT):
        x0 = float(np.float32(1.0 + (i + 0.5) * width))
        y = _g_derivs(w, (x0 + cheb - 1.0) * 8.0, 0)
        c = np.polyfit(cheb, y, 3)
        ent[i, 0] = c[3]
        ent[i, 1] = c[2]
        ent[i, 2] = c[1]
        ent[i, 3] = c[0]
        ent[i, 4] = x0
    ent[NBKT, 0] = _g_derivs(w, 0.0, 0)
    ent[NBKT, 4] = 2.0
    return ent


def _patch_pwp_dir(w):
    """Copy the arch pwp dir and rewrite the silu table of silu_and_others
    with 512 cubic buckets of G(x) = g(x-1) on [1,2). Returns (dir, digest)."""
    from neuronxcc.driver.Job import Job
    from neuronxcc.driver.jobs.support.FindActInfo import findActInfoFile

    src = os.path.dirname(findActInfoFile(Job.getPackageDir(), "gen3"))
    ent = _build_bucket_entries(w)
    digest = hashlib.sha256(
        ent.tobytes() + struct.pack("<II", NBKT_LOG2, SILU_CTL_EXP0)
    ).hexdigest()[:12]
    dst = os.path.join(tempfile.gettempdir(), f"pwp_g_{digest}")
    if not os.path.isdir(dst):
        tmp = dst + f".tmp{os.getpid()}"
        if os.path.isdir(tmp):
            shutil.rmtree(tmp)
        shutil.copytree(src, tmp)
        for f in os.listdir(tmp):
            os.chmod(os.path.join(tmp, f), 0o644)
        bkt_path = os.path.join(tmp, "silu_and_others_bkt.bin")
        bkt = bytearray(open(bkt_path, "rb").read())
        bkt[0 : (NBKT + 1) * 32] = ent.tobytes()
        open(bkt_path, "wb").write(bytes(bkt))
        ctl_path = os.path.join(tmp, "silu_and_others_ctrl.bin")
        ctl = bytearray(open(ctl_path, "rb").read())
        w0 = (NBKT_LOG2 << 16) | ((23 - NBKT_LOG2) << 11) | 0
        ctl[SILU_CTL_EXP0 * 32 : SILU_CTL_EXP0 * 32 + 4] = struct.pack("<I", w0)
        wc = (0 << 16) | (23 << 11) | NBKT  # const bucket, exps 1..3
        for e in range(1, 4):
            ctl[(SILU_CTL_EXP0 + e) * 32 : (SILU_CTL_EXP0 + e) * 32 + 4] = (
                struct.pack("<I", wc)
            )
        open(ctl_path, "wb").write(bytes(ctl))
        os.rename(tmp, dst)
    return dst, digest


# --------------------------------------------------------------- bass build

def _build_nc(digest):
    import concourse.bacc as bacc
    import concourse.mybir as mybir
    import concourse.tile as tile
    import concourse.bass as bass

    f32 = mybir.dt.float32
    f16 = mybir.dt.float16
    bf16 = mybir.dt.bfloat16
    Act = mybir.ActivationFunctionType
    Alu = mybir.AluOpType

    nc = bacc.Bacc(
        "TRN2",
        target_bir_lowering=False,
        debug=False,
        num_devices=NCORES,
    )

    # z/q/n2 operand blocks are packed along the free dim so every matmul
    # operand slice starts at a legal SBUF partition base (0/32/64): z at
    # [0:44, block0], q at [0:24, block1], n2 at [32:45, block1].
    noise_d = nc.dram_tensor("noise", [NPC, P, FD], bf16, kind="ExternalInput")
    lhs_d = nc.dram_tensor("lhs", [NPC, KR, 2 * P], f16, kind="ExternalInput")
    rhs_d = nc.dram_tensor("rhs", [KR, 2 * FD], f16, kind="ExternalInput")
    # table-digest marker: forces a distinct BIR (and thus NEFF cache key)
    # per activation-table contents.
    dig_d = nc.dram_tensor(f"tdig_{digest}", [1, 8], f32, kind="ExternalInput")
    out_d = nc.dram_tensor("out", [NPC, P, FD], bf16, kind="ExternalOutput")

    with tile.TileContext(nc) as tc:
        with (
            tc.tile_pool(name="const", bufs=1) as cpool,
            tc.tile_pool(name="work", bufs=3) as pool,
            tc.tile_pool(name="psum", bufs=2, space=bass.MemorySpace.PSUM) as psum,
        ):
            # compute-gating operands (rhs, lhs) go on the gpsimd DMA queue,
            # bulk noise on the Sync queue; the Scalar queue stays DMA-free
            # so the act-table load is emitted exactly once, early.
            rhs_t = cpool.tile([KR, 2 * FD], f16)
            nc.gpsimd.dma_start(rhs_t[:], rhs_d[:])
            dig_t = cpool.tile([1, 8], f32)
            lhs_ts = []
            noise_ts = []
            for n in range(NPC):
                lhs_ts.append(cpool.tile([KR, 2 * P], f16, name=f"lhs{n}"))
                nc.gpsimd.dma_start(lhs_ts[n][:], lhs_d[n])
                noise_ts.append(cpool.tile([P, FD], bf16, name=f"noise{n}"))
                nc.sync.dma_start(noise_ts[n][:], noise_d[n])
            nc.sync.dma_start(dig_t[:], dig_d[:])

            HF = 512  # pipeline-unit free dim (one PSUM bank)
            for n in range(NPC):
                lhs_t = lhs_ts[n]
                noise_t = noise_ts[n]
                o_t = pool.tile([P, FD], bf16, tag="o")

                for half in range(2):
                    cols = bass.ts(half, HF)

                    # z = phase cycles; vmsq = vm^2; n2 = alpha + beta*vm
                    z_p = psum.tile([P, HF], f32, tag="z")
                    nc.tensor.matmul(
                        z_p[:], lhs_t[0:KZ, 0:P], rhs_t[0:KZ, cols]
                    )
                    q_p = psum.tile([P, HF], f32, tag="q")
                    nc.tensor.matmul(
                        q_p[:],
                        lhs_t[0:KQ, P : 2 * P],
                        rhs_t[0:KQ, bass.ts(2 + half, HF)],
                    )
                    n2_p = psum.tile([P, HF], f32, tag="n2")
                    nc.tensor.matmul(
                        n2_p[:],
                        lhs_t[32 : 32 + KN, P : 2 * P],
                        rhs_t[32 : 32 + KN, bass.ts(2 + half, HF)],
                    )

                    # g(z mod 1) via the patched silu table: silu(z/8 + 1)
                    g_t = pool.tile([P, HF], bf16, tag="g")
                    nc.scalar.activation(
                        g_t[:], z_p[:], Act.Silu, bias=1.0, scale=0.125
                    )

                    # pre = g*vmsq + noise*n2 (in place: g <- g*q;
                    # noise <- noise*n2; g <- g + noise)
                    nc.vector.tensor_mul(g_t[:], g_t[:], q_p[:])
                    nc.vector.tensor_mul(
                        noise_t[:, cols], noise_t[:, cols], n2_p[:]
                    )
                    nc.gpsimd.tensor_add(g_t[:], g_t[:], noise_t[:, cols])
                    nc.scalar.activation(o_t[:, cols], g_t[:], Act.Tanh)
                nc.sync.dma_start(out_d[n], o_t[:])

    nc.compile()
    return nc


# ------------------------------------------------------------------- driver

def _host_inputs(f0, uv, weight, noise, alpha, beta):
    """Build the per-core input maps (all host math in f64)."""
    import ml_dtypes

    f0 = np.asarray(f0, np.float64).reshape(N, L)
    uv = np.asarray(uv, np.float64).reshape(N, L)
    noise_bf = np.ascontiguousarray(
        np.asarray(noise, np.float32).reshape(N, T)
    ).astype(ml_dtypes.bfloat16)

    c1, c2, c3, A, B, C = _interp_consts()
    Fp, Fc, Fn = _neighbors(f0 / SR)
    Up, Uc, Un = _neighbors(uv)

    # per-frame phase offsets (cycles), f64 exact then frac
    FS = Fp * A[-1] + Fc * B[-1] + Fn * C[-1]  # frame sums of f0_up/SR
    C0 = np.concatenate([np.zeros((N, 1)), np.cumsum(FS, axis=1)[:, :-1]], axis=1)
    D1 = np.mod(C0, 1.0)

    A1, A2 = _f16_split(A)
    B1, B2 = _f16_split(B)
    C1v, C2v = _f16_split(C)
    ones = np.ones(FRAME)

    # rhs rows per seg: [A1,B1,C1, A2,B2,C2, A1,B1,C1, 1, 1] pairing with
    # lhs  rows        [F1p,F1c,F1n, F1p,F1c,F1n, F2p,F2c,F2n, D11, D12]
    rhs_z = _windowed_rhs(
        [A1, B1, C1v, A2, B2, C2v, A1, B1, C1v, ones, ones]
    ).astype(np.float16)
    # vm^2 quadratic form: rhs [c1^2,c2^2,c3^2, c1c2,c1c3,c2c3] against
    # lhs [Up^2,Uc^2,Un^2, 2UpUc,2UpUn,2UcUn]
    rhs_q = _windowed_rhs(
        [c1 * c1, c2 * c2, c3 * c3, c1 * c2, c1 * c3, c2 * c3]
    ).astype(np.float16)
    # n2 = alpha + beta*vm: windowed beta*c rows + one global alpha row
    rhs_n2 = np.concatenate(
        [
            _windowed_rhs([beta * c1, beta * c2, beta * c3]),
            np.full((1, FD), alpha),
        ]
    ).astype(np.float16)

    F1p, F2p = _f16_split(Fp)
    F1c, F2c = _f16_split(Fc)
    F1n, F2n = _f16_split(Fn)
    D11, D12 = _f16_split(D1)

    # pack z/q/n2 blocks along the free dim (n2 at partition base 32)
    rhs = np.zeros((KR, 2 * FD), dtype=np.float16)
    rhs[:KZ, 0:FD] = rhs_z
    rhs[:KQ, FD : 2 * FD] = rhs_q
    rhs[32 : 32 + KN, FD : 2 * FD] = rhs_n2

    jidx = 4 * np.arange(P)[None, :] + np.arange(SEG)[:, None]  # [SEG, P]
    zrows = [F1p, F1c, F1n, F1p, F1c, F1n, F2p, F2c, F2n, D11, D12]
    qrows = [Up * Up, Uc * Uc, Un * Un, 2 * Up * Uc, 2 * Up * Un, 2 * Uc * Un]
    nrows = [Up, Uc, Un]
    in_maps = []
    for core in range(NCORES):
        rows = range(core * NPC, (core + 1) * NPC)
        lhs = np.zeros((NPC, KR, 2 * P), dtype=np.float16)
        lhs[:, 32 + KN - 1, P : 2 * P] = 1.0  # alpha row
        for ln, nr in enumerate(rows):
            for seg in range(SEG):
                j = jidx[seg]
                for i, r in enumerate(zrows):
                    lhs[ln, seg * 11 + i, 0:P] = r[nr, j].astype(np.float16)
                for i, r in enumerate(qrows):
                    lhs[ln, seg * 6 + i, P : 2 * P] = r[nr, j].astype(np.float16)
                for i, r in enumerate(nrows):
                    lhs[ln, 32 + seg * 3 + i, P : 2 * P] = r[nr, j].astype(
                        np.float16
                    )
        in_maps.append(
            {
                "noise": noise_bf[core * NPC : (core + 1) * NPC].reshape(NPC, P, FD),
                "lhs": lhs,
                "rhs": rhs,
            }
        )
    return in_maps


def kernel(f0, uv, weight, noise):
    global LAST_RESULTS
    from concourse.bass_utils import run_bass_kernel_spmd

    weight = np.asarray(weight, np.float64).reshape(NH)
    w = np.exp(weight)
    w = w / max(np.sqrt((w * w).sum()), 1e-12)
    S = float(w.sum())
    alpha = float(np.float32(0.333 * S))
    beta = float(np.float32((0.2 - 0.333) * S))

    pwp_dir, digest = _patch_pwp_dir(w)
    os.environ["BASS_ACT_ROOT_JSON_PATH"] = os.path.join(pwp_dir, "act_info.json")

    key = digest
    if key not in _NC_CACHE:
        _NC_CACHE[key] = _build_nc(digest)
    nc = _NC_CACHE[key]

    in_maps = _host_inputs(f0, uv, weight, noise, alpha, beta)
    dig = np.zeros((1, 8), dtype=np.float32)
    dig[0, :] = np.frombuffer(
        hashlib.sha256(digest.encode()).digest()[:32], dtype=np.float32
    )[:8]
    for m in in_maps:
        m[f"tdig_{digest}"] = dig

    res = run_bass_kernel_spmd(nc, in_maps, list(range(NCORES)))
    LAST_RESULTS = res
    out = np.empty((N, 1, T), dtype=np.float32)
    for core in range(NCORES):
        out[core * NPC : (core + 1) * NPC, 0, :] = (
            res.results[core]["out"].astype(np.float32).reshape(NPC, T)
        )
    return out
